# revision 42
# baseline (speedup 1.0000x reference)
"""Trainium2 Bass kernel for sparse transposed conv (gather-GEMM-scatter + ReLU).

Strategy: exact-compute grouped GEMM over class-sorted parents. Each output
row j equals relu(feats[parent(j)] @ weight[koff(j)]) for exactly one
(parent, koff) pair, and each parent matches exactly 4 of the 8 kernel
offsets. The host sorts parents by their 4-offset "class" (70 possible
4-subsets), ordered along a revolving-door Gray code -- a Hamiltonian path
on the Johnson graph J(8,4) -- so that for every offset k the matched
parents form only ~9 contiguous runs (73 total across the 8 offsets). The
device then runs, per offset, plain <=512-wide bf16 matmuls over those
contiguous column ranges: zero data-dependent addressing, no GPSIMD
gathers (the original kernel's ap_gather cost ~33ns/index = ~3.4ms total;
this design's device program is gather-free), and no wasted FLOPs (only
the ~50k matched tokens per core are computed).

Sharding: parents are dealt per-class round-robin across the 8 cores
(member m of class g -> core m%8, padded slot off[g] + m//8), so per-core
class counts differ by <=1 and one SPMD program with a shared padded
layout serves all cores at ~0.3% padding. The host-side unshard picks,
for each output row, its token from the owning core's result (pure numpy
fancy-index inverse permutation).

Device pipeline per core (~64-70us measured):
  - x chunks ([h0 block | h1 block] per chunk: one contiguous DMA line
    per partition AND unstrided rhs slices) staggered just-in-time into
    the block stream, because the 16 SDMA queues serve active DMAs
    round-robin and anything issued together finishes together.
  - ~3us of dummy matmuls warm the PE HAM clock gate to 8/8 first.
  - Per 512-token PSUM block: 2 accumulating matmuls per piece (C_in =
    2x128 contraction halves), ReLU + f32->bf16 fused into the PSUM drain,
    alternating ScalarE/VectorE; 8-block staging tiles DMA to HBM k-major.
Per-core DMA: 6.4MB in + 12.85MB out (both at the bf16 information
floor); PE streaming floor ~42us; measured PE-dense window ~46us.
"""

import functools
import os

import numpy as np

N_IN = 100_000
K = 8
C_IN = 256
C_OUT = 128
CHILDREN = 4
N_OUT = N_IN * CHILDREN
NCORES = 8
R = N_IN // NCORES        # feats rows per core (12500)
PB = 512                  # tokens per PSUM block (= one f32 bank)
YB = 8                    # PSUM blocks per output staging tile / DMA

LAST_RESULTS = None       # test.py reads exec_time_ns from here


def _revdoor(n, k):
    """Revolving-door Gray code: all k-subsets of range(n), consecutive
    subsets differing by exactly one swap (Hamiltonian path on J(n,k))."""
    if k == 0:
        return [[]]
    if k == n:
        return [list(range(n))]
    return _revdoor(n - 1, k) + [c + [n - 1]
                                 for c in reversed(_revdoor(n - 1, k - 1))]


_CLASS_MASKS = [sum(1 << x for x in c) for c in _revdoor(K, CHILDREN)]
_RANK_OF_MASK = {m: i for i, m in enumerate(_CLASS_MASKS)}
NCLS = len(_CLASS_MASKS)  # 70


def _layout(cnt_max):
    """Shared (all-core) padded layout derived from per-class max counts.

    Tokens are ordered CHUNK-major (all 8 offsets' ranges within x chunk 0
    first, then chunk 1, ...) so the PE only ever needs already-DMA'd x
    data: the first chunk is small to start the PE early, later chunks
    stream in well ahead of consumption. Returns (NP, off, bounds, pieces,
    T) where pieces is the ordered list of (k, chunk, local_off, tok, n)
    and each piece fits within one x chunk and one 512-token PSUM block.
    """
    off = np.zeros(NCLS + 1, dtype=np.int64)
    off[1:] = np.cumsum(cnt_max)
    NP = int(off[NCLS])
    big = -(-(NP - 2816) // 2)
    bounds = [0, 256, 1024, 2816, 2816 + big, NP]
    bounds = [min(b, NP) for b in bounds]
    assert all(bounds[i] < bounds[i + 1] for i in range(len(bounds) - 1))
    runs = []
    for k in range(K):
        i = 0
        while i < NCLS:
            if (_CLASS_MASKS[i] >> k) & 1 and cnt_max[i] > 0:
                j = i
                while j < NCLS and (_CLASS_MASKS[j] >> k) & 1:
                    j += 1
                runs.append((k, int(off[i]), int(off[j])))
                i = j
            else:
                i += 1
    pieces = []
    tok = 0
    for c in range(len(bounds) - 1):
        lo, hi = bounds[c], bounds[c + 1]
        for k in range(K):
            for rk, ra, rb in runs:
                if rk != k:
                    continue
                a, b = max(ra, lo), min(rb, hi)
                x = a
                while x < b:
                    take = min(b - x, PB - (tok % PB))
                    pieces.append((k, c, x - lo, tok, take))
                    tok += take
                    x += take
    assert tok == sum(rb - ra for _, ra, rb in runs)
    return NP, off, bounds, pieces, tok


@functools.lru_cache(maxsize=2)
def _build_program(cnt_key):
    from contextlib import ExitStack

    import concourse.tile as tile
    from concourse import bacc, mybir

    F32 = mybir.dt.float32
    BF16 = mybir.dt.bfloat16

    cnt_max = np.asarray(cnt_key, dtype=np.int64)
    NP, off, bounds, pieces, T = _layout(cnt_max)
    T512 = -(-T // PB) * PB
    if T512 > T:
        pieces = pieces + [(0, 0, 0, T, T512 - T)]  # filler fills last bank
    nblocks = T512 // PB
    blocks = [[] for _ in range(nblocks)]
    for k, ch, loff, tok, n in pieces:
        blocks[tok // PB].append((k, ch, loff, tok % PB, n))

    nc = bacc.Bacc("TRN2", target_bir_lowering=False, debug=False,
                   num_devices=NCORES)
    # x chunk c occupies columns [2*a, 2*b): first the h=0 half-rows
    # (channels p), then the h=1 half-rows (channels 128+p). One contiguous
    # DMA line per partition per chunk AND unstrided matmul rhs slices.
    x_d = nc.dram_tensor("x", [128, 2 * NP], BF16, kind="ExternalInput").ap()
    # w[p, g, (k%4)*2+h, co] = weight[4*g + k%4, h*128 + p, co]
    w_d = nc.dram_tensor("w", [128, 2, K, C_OUT], BF16,
                         kind="ExternalInput").ap()
    out_d = nc.dram_tensor("out", [128, T512], BF16,
                           kind="ExternalOutput").ap()

    with tile.TileContext(nc) as tc, ExitStack() as ctx:
        cpool = ctx.enter_context(tc.tile_pool(name="const", bufs=2))
        w_lo = cpool.tile([128, K, C_OUT], BF16)
        w_hi = cpool.tile([128, K, C_OUT], BF16)
        nc.sync.dma_start(out=w_lo[:], in_=w_d[:, 0])

        # Distinct buffers for every x chunk (no aliasing: an aliased chunk
        # DMA would make the in-order Sync engine block all later y-output
        # DMA issuance on its consumption wait). The 16 SDMA queues serve
        # all active DMAs round-robin, so anything issued together finishes
        # together: each chunk's dma_start is therefore staggered into the
        # block stream below (just-in-time), keeping early chunks from
        # being starved and leaving late bandwidth for the output drain.
        nch = len(bounds) - 1
        xpool_s = ctx.enter_context(tc.tile_pool(name="xs", bufs=2))
        xpool_b = ctx.enter_context(tc.tile_pool(name="xb", bufs=3))
        xts = []
        for c in range(nch):
            a, b = bounds[c], bounds[c + 1]
            pool = xpool_s if c < 2 else xpool_b
            xt = pool.tile([128, 2 * (b - a)], BF16)
            if c < 3:
                nc.sync.dma_start(out=xt[:], in_=x_d[:, 2 * a:2 * b])
            if c == 0:
                # w_hi is only needed from block 1 on: issue it after chunk
                # 0 so block 0's operands get the early queue bandwidth
                nc.sync.dma_start(out=w_hi[:], in_=w_d[:, 1])
            xts.append(xt)

        ypool = ctx.enter_context(tc.tile_pool(name="y", bufs=8))
        psmm = ctx.enter_context(tc.tile_pool(name="ps", bufs=8,
                                              space="PSUM"))

        dummy = cpool.tile([128, 128], BF16)
        nc.vector.memset(dummy[:], 0.0)

        # group sizes: YB blocks, but small groups at the end to shrink the
        # final relu->DMA tail
        groups = []
        rem = nblocks
        while rem > 12:
            groups.append(YB)
            rem -= YB
        while rem > 0:
            groups.append(min(2, rem))
            rem -= min(2, rem)
        # just-in-time x chunk issuance points (after group gi's y DMA)
        xsched = {0: 3, 2: 4}

        bb0 = 0
        for gi, nb in enumerate(groups):
            y = ypool.tile([128, nb * PB], BF16)
            for bb in range(bb0, bb0 + nb):
                ps = psmm.tile([128, PB], F32)
                if bb == 0:
                    # PE warm-up: ~3us of dummy matmuls so the HAM clock
                    # gate reaches 8/8 before the first real matmul; the
                    # real pieces below overwrite (start=True).
                    for _ in range(28):
                        nc.tensor.matmul(out=ps[:, :128], lhsT=dummy[:],
                                         rhs=dummy[:], start=True, stop=True)
                for k, ch, loff, col0, n in blocks[bb]:
                    wc = bounds[ch + 1] - bounds[ch]
                    nc.tensor.matmul(
                        out=ps[:, col0:col0 + n],
                        lhsT=w_lo[:, k, :] if k < 4 else w_hi[:, k - 4, :],
                        rhs=xts[ch][:, loff:loff + n],
                        start=True, stop=False)
                    nc.tensor.matmul(
                        out=ps[:, col0:col0 + n],
                        lhsT=w_lo[:, k + 4, :] if k < 4
                        else w_hi[:, k, :],
                        rhs=xts[ch][:, wc + loff:wc + loff + n],
                        start=False, stop=True)
                # ReLU + f32->bf16 on the PSUM drain; alternate engines
                dst = y[:, (bb - bb0) * PB:(bb - bb0 + 1) * PB]
                if bb % 2 == 0:
                    nc.scalar.activation(
                        out=dst, in_=ps[:],
                        func=mybir.ActivationFunctionType.Relu)
                else:
                    nc.vector.tensor_scalar_max(dst, ps[:], 0.0)
            nc.sync.dma_start(
                out=out_d[:, bb0 * PB:(bb0 + nb) * PB], in_=y[:])
            xc = xsched.get(gi)
            if xc is not None:
                nc.sync.dma_start(
                    out=xts[xc][:],
                    in_=x_d[:, 2 * bounds[xc]:2 * bounds[xc + 1]])
            bb0 += nb

    nc.compile()
    return nc


def _ensure_ntff_hook():
    """This image's antenv lacks axon_hooks; synthesize it so trace=True can
    drive NTFF profiling via the injected libaxon_pjrt.so."""
    import sys
    import types
    try:
        import antenv.axon_hooks  # noqa: F401
        return True
    except ImportError:
        pass
    try:
        import antenv
        from trn_agent_boot.trn_boot import _ntff_profile_via_ctypes
    except ImportError:
        return False
    mod = types.ModuleType("antenv.axon_hooks")
    holder = {}
    mod.set_axon_ntff_profile_hook = lambda h: holder.__setitem__("h", h)
    mod.get_axon_ntff_profile_hook = lambda: holder.get("h")
    sys.modules["antenv.axon_hooks"] = mod
    antenv.axon_hooks = mod
    try:
        h = _ntff_profile_via_ctypes("/opt/axon/libaxon_pjrt.so")
    except OSError:
        h = None
    if h is not None:
        mod.set_axon_ntff_profile_hook(h)
    return True


def kernel(**inputs):
    global LAST_RESULTS
    import ml_dtypes
    from concourse.bass_utils import run_bass_kernel_spmd

    bf16 = ml_dtypes.bfloat16
    feats = np.asarray(inputs["feats"], dtype=np.float32)
    weight = np.asarray(inputs["weight"], dtype=np.float32)
    gather_idx = np.asarray(inputs["gather_idx"], dtype=np.int64)
    scatter_idx = np.asarray(inputs["scatter_idx"], dtype=np.int64)
    n_out = int(inputs["n_out"])
    assert feats.shape == (N_IN, C_IN) and weight.shape == (K, C_IN, C_OUT)
    assert n_out == N_OUT

    # Per output row j: its unique (parent, koff) match from the match lists.
    par_j = np.zeros(N_OUT, dtype=np.int64)
    koff_j = np.zeros(N_OUT, dtype=np.int64)
    covered = np.zeros(N_OUT, dtype=bool)
    for k in range(K):
        s = scatter_idx[k]
        g = gather_idx[k]
        valid = (s < N_OUT) & (g < N_IN)
        par_j[s[valid]] = g[valid]
        koff_j[s[valid]] = k
        covered[s[valid]] = True

    # Class of each parent = bitmask of its matched offsets (exactly 4 set).
    cls = np.zeros(N_IN, dtype=np.int64)
    np.bitwise_or.at(cls, par_j[covered], np.int64(1) << koff_j[covered])
    popc = np.zeros(N_IN, dtype=np.int64)
    for k in range(K):
        popc += (cls >> k) & 1
    assert (popc == CHILDREN).all(), "every parent must match exactly 4 offsets"
    lut = np.full(256, -1, dtype=np.int64)
    for i, m in enumerate(_CLASS_MASKS):
        lut[m] = i
    crank = lut[cls]
    assert (crank >= 0).all()

    # Shard parents per-class round-robin across cores: member m of class g
    # goes to core m%8 at padded slot off[g] + m//8, so per-core class
    # counts differ by at most 1 and the shared padded layout wastes ~0.3%
    # instead of ~10% (core-range sharding). The host-side selection below
    # may read any core's slab, so sharding is free to permute parents.
    order_g = np.argsort(crank, kind="stable")
    sorted_ranks = crank[order_g]
    n_g = np.bincount(crank, minlength=NCLS)
    grp_start = np.zeros(NCLS, dtype=np.int64)
    grp_start[1:] = np.cumsum(n_g)[:-1]
    m_idx = np.arange(N_IN) - grp_start[sorted_ranks]
    core_of = np.empty(N_IN, dtype=np.int64)
    core_of[order_g] = m_idx % NCORES
    cnt_max = -(-n_g // NCORES)
    NP, off, bounds, pieces, T = _layout(cnt_max)
    T512 = -(-T // PB) * PB
    pp_all = np.empty(N_IN, dtype=np.int64)
    pp_all[order_g] = off[sorted_ranks] + m_idx // NCORES

    # Token index of every padded x slot, per offset (device piece order);
    # identical for all cores.
    tokmap = np.full((K, NP), -1, dtype=np.int64)
    for k, ch, loff, tok, n in pieces:
        xoff = bounds[ch] + loff
        tokmap[k, xoff:xoff + n] = np.arange(tok, tok + n)

    # Per-core bf16 operand layout.
    w2 = np.ascontiguousarray(
        weight.reshape(2, 4, 2, 128, C_OUT).transpose(3, 0, 2, 1, 4)
    ).reshape(128, 2, K, C_OUT).astype(bf16)
    in_maps = []
    for c in range(NCORES):
        mine = core_of == c
        f = np.zeros((NP, C_IN), dtype=np.float32)
        f[pp_all[mine]] = feats[mine]
        fh = f.reshape(NP, 2, 128).transpose(2, 1, 0)   # [p, h, i]
        x = np.empty((128, 2 * NP), dtype=np.float32)
        for a, b in zip(bounds[:-1], bounds[1:]):
            x[:, 2 * a:a + b] = fh[:, 0, a:b]
            x[:, a + b:2 * b] = fh[:, 1, a:b]
        in_maps.append({"x": x.astype(bf16), "w": w2})

    nc = _build_program(tuple(int(v) for v in cnt_max))
    trace = bool(int(os.environ.get("KERNEL_TRACE", "0")))
    if trace:
        trace = _ensure_ntff_hook()
    res = run_bass_kernel_spmd(nc, in_maps, list(range(NCORES)), trace=trace)
    LAST_RESULTS = res

    # Unshard: token -> output row inverse permutation (pure numpy).
    a_all = np.stack([np.asarray(res.results[c]["out"])
                      for c in range(NCORES)])          # [8, 128, T512] bf16
    out = np.zeros((N_OUT, C_OUT), dtype=np.float32)
    pj = par_j[covered]
    tok = tokmap[koff_j[covered], pp_all[pj]]
    assert (tok >= 0).all()
    out[covered] = a_all[core_of[pj], :, tok].astype(np.float32)
    return out


# revision 43
# speedup vs baseline: 1.1620x; 1.1620x over previous
"""Trainium2 Bass kernel for sparse transposed conv (gather-GEMM-scatter + ReLU).

Strategy: exact-compute grouped GEMM over class-sorted parents. Each output
row j equals relu(feats[parent(j)] @ weight[koff(j)]) for exactly one
(parent, koff) pair, and each parent matches exactly 4 of the 8 kernel
offsets. The host sorts parents by their 4-offset "class" (70 possible
4-subsets), ordered along a revolving-door Gray code -- a Hamiltonian path
on the Johnson graph J(8,4) -- so that for every offset k the matched
parents form only ~9 contiguous runs (73 total across the 8 offsets). The
device then runs, per offset, plain <=512-wide bf16 matmuls over those
contiguous column ranges: zero data-dependent addressing, no GPSIMD
gathers (the original kernel's ap_gather cost ~33ns/index = ~3.4ms total;
this design's device program is gather-free), and no wasted FLOPs (only
the ~50k matched tokens per core are computed).

Sharding: parents are dealt per-class round-robin across the 8 cores
(member m of class g -> core m%8, padded slot off[g] + m//8), so per-core
class counts differ by <=1 and one SPMD program with a shared padded
layout serves all cores at ~0.3% padding. The host-side unshard picks,
for each output row, its token from the owning core's result (pure numpy
fancy-index inverse permutation).

Device pipeline per core (~64-70us measured):
  - x chunks ([h0 block | h1 block] per chunk: one contiguous DMA line
    per partition AND unstrided rhs slices) staggered just-in-time into
    the block stream, because the 16 SDMA queues serve active DMAs
    round-robin and anything issued together finishes together.
  - ~3us of dummy matmuls warm the PE HAM clock gate to 8/8 first.
  - Per 512-token PSUM block: 2 accumulating matmuls per piece (C_in =
    2x128 contraction halves), ReLU + f32->bf16 fused into the PSUM drain,
    alternating ScalarE/VectorE; 8-block staging tiles DMA to HBM k-major.
Per-core DMA: 6.4MB in + 12.85MB out (both at the bf16 information
floor); PE streaming floor ~42us; measured PE-dense window ~46us.
"""

import functools
import os

import numpy as np

N_IN = 100_000
K = 8
C_IN = 256
C_OUT = 128
CHILDREN = 4
N_OUT = N_IN * CHILDREN
NCORES = 8
R = N_IN // NCORES        # feats rows per core (12500)
PB = 512                  # tokens per PSUM block (= one f32 bank)
YB = 8                    # PSUM blocks per output staging tile / DMA

LAST_RESULTS = None       # test.py reads exec_time_ns from here


def _revdoor(n, k):
    """Revolving-door Gray code: all k-subsets of range(n), consecutive
    subsets differing by exactly one swap (Hamiltonian path on J(n,k))."""
    if k == 0:
        return [[]]
    if k == n:
        return [list(range(n))]
    return _revdoor(n - 1, k) + [c + [n - 1]
                                 for c in reversed(_revdoor(n - 1, k - 1))]


_CLASS_MASKS = [sum(1 << x for x in c) for c in _revdoor(K, CHILDREN)]
_RANK_OF_MASK = {m: i for i, m in enumerate(_CLASS_MASKS)}
NCLS = len(_CLASS_MASKS)  # 70


def _layout(cnt_max):
    """Shared (all-core) padded layout derived from per-class max counts.

    Tokens are ordered CHUNK-major (all 8 offsets' ranges within x chunk 0
    first, then chunk 1, ...) so the PE only ever needs already-DMA'd x
    data: the first chunk is small to start the PE early, later chunks
    stream in well ahead of consumption. Returns (NP, off, bounds, pieces,
    T) where pieces is the ordered list of (k, chunk, local_off, tok, n)
    and each piece fits within one x chunk and one 512-token PSUM block.
    """
    off = np.zeros(NCLS + 1, dtype=np.int64)
    off[1:] = np.cumsum(cnt_max)
    NP = int(off[NCLS])
    big = -(-(NP - 2816) // 2)
    bounds = [0, 256, 1024, 2816, 2816 + big, NP]
    bounds = [min(b, NP) for b in bounds]
    assert all(bounds[i] < bounds[i + 1] for i in range(len(bounds) - 1))
    runs = []
    for k in range(K):
        i = 0
        while i < NCLS:
            if (_CLASS_MASKS[i] >> k) & 1 and cnt_max[i] > 0:
                j = i
                while j < NCLS and (_CLASS_MASKS[j] >> k) & 1:
                    j += 1
                runs.append((k, int(off[i]), int(off[j])))
                i = j
            else:
                i += 1
    pieces = []
    tok = 0
    for c in range(len(bounds) - 1):
        lo, hi = bounds[c], bounds[c + 1]
        for k in range(K):
            for rk, ra, rb in runs:
                if rk != k:
                    continue
                a, b = max(ra, lo), min(rb, hi)
                x = a
                while x < b:
                    take = min(b - x, PB - (tok % PB))
                    pieces.append((k, c, x - lo, tok, take))
                    tok += take
                    x += take
    assert tok == sum(rb - ra for _, ra, rb in runs)
    return NP, off, bounds, pieces, tok


@functools.lru_cache(maxsize=2)
def _build_program(cnt_key):
    from contextlib import ExitStack

    import concourse.tile as tile
    from concourse import bacc, mybir

    F32 = mybir.dt.float32
    BF16 = mybir.dt.bfloat16

    cnt_max = np.asarray(cnt_key, dtype=np.int64)
    NP, off, bounds, pieces, T = _layout(cnt_max)
    T512 = -(-T // PB) * PB
    if T512 > T:
        pieces = pieces + [(0, 0, 0, T, T512 - T)]  # filler fills last bank
    nblocks = T512 // PB
    blocks = [[] for _ in range(nblocks)]
    for k, ch, loff, tok, n in pieces:
        blocks[tok // PB].append((k, ch, loff, tok % PB, n))

    nc = bacc.Bacc("TRN2", target_bir_lowering=False, debug=False,
                   num_devices=NCORES)
    # x chunk c occupies columns [2*a, 2*b): first the h=0 half-rows
    # (channels p), then the h=1 half-rows (channels 128+p). One contiguous
    # DMA line per partition per chunk AND unstrided matmul rhs slices.
    x_d = nc.dram_tensor("x", [128, 2 * NP], BF16, kind="ExternalInput").ap()
    # w[p, g, (k%4)*2+h, co] = weight[4*g + k%4, h*128 + p, co]
    w_d = nc.dram_tensor("w", [128, 2, K, C_OUT], BF16,
                         kind="ExternalInput").ap()
    out_d = nc.dram_tensor("out", [128, T512], BF16,
                           kind="ExternalOutput").ap()

    with tile.TileContext(nc) as tc, ExitStack() as ctx:
        cpool = ctx.enter_context(tc.tile_pool(name="const", bufs=2))
        w_lo = cpool.tile([128, K, C_OUT], BF16)
        w_hi = cpool.tile([128, K, C_OUT], BF16)
        nc.sync.dma_start(out=w_lo[:], in_=w_d[:, 0])
        nc.sync.dma_start(out=w_hi[:], in_=w_d[:, 1])

        # Distinct buffers for every x chunk (no aliasing: an aliased chunk
        # DMA would make the in-order Sync engine block all later y-output
        # DMA issuance on its consumption wait). The 16 SDMA queues serve
        # all active DMAs round-robin, so anything issued together finishes
        # together: each chunk's dma_start is therefore staggered into the
        # block stream below (just-in-time), keeping early chunks from
        # being starved and leaving late bandwidth for the output drain.
        nch = len(bounds) - 1
        xpool_s = ctx.enter_context(tc.tile_pool(name="xs", bufs=2))
        xpool_b = ctx.enter_context(tc.tile_pool(name="xb", bufs=3))
        xts = []
        for c in range(nch):
            a, b = bounds[c], bounds[c + 1]
            pool = xpool_s if c < 2 else xpool_b
            xt = pool.tile([128, 2 * (b - a)], BF16)
            if c < 3:
                nc.sync.dma_start(out=xt[:], in_=x_d[:, 2 * a:2 * b])
            xts.append(xt)

        ypool = ctx.enter_context(tc.tile_pool(name="y", bufs=6))
        psmm = ctx.enter_context(tc.tile_pool(name="ps", bufs=8,
                                              space="PSUM"))

        dummy = cpool.tile([128, 128], BF16)
        nc.vector.memset(dummy[:], 0.0)

        # group sizes: YB blocks, but small groups at the end to shrink the
        # final relu->DMA tail
        groups = []
        rem = nblocks
        while rem > 12:
            groups.append(YB)
            rem -= YB
        while rem > 0:
            groups.append(min(2, rem))
            rem -= min(2, rem)
        # just-in-time x chunk issuance points (after group gi's y DMA)
        xsched = {0: 3, 2: 4}

        bb0 = 0
        for gi, nb in enumerate(groups):
            y = ypool.tile([128, nb * PB], BF16)
            for bb in range(bb0, bb0 + nb):
                ps = psmm.tile([128, PB], F32)
                if bb == 0:
                    # PE warm-up: ~3us of dummy matmuls so the HAM clock
                    # gate reaches 8/8 before the first real matmul; the
                    # real pieces below overwrite (start=True).
                    for _ in range(28):
                        nc.tensor.matmul(out=ps[:, :128], lhsT=dummy[:],
                                         rhs=dummy[:], start=True, stop=True)
                for k, ch, loff, col0, n in blocks[bb]:
                    wc = bounds[ch + 1] - bounds[ch]
                    nc.tensor.matmul(
                        out=ps[:, col0:col0 + n],
                        lhsT=w_lo[:, k, :] if k < 4 else w_hi[:, k - 4, :],
                        rhs=xts[ch][:, loff:loff + n],
                        start=True, stop=False)
                    nc.tensor.matmul(
                        out=ps[:, col0:col0 + n],
                        lhsT=w_lo[:, k + 4, :] if k < 4
                        else w_hi[:, k, :],
                        rhs=xts[ch][:, wc + loff:wc + loff + n],
                        start=False, stop=True)
                # ReLU + f32->bf16 on the PSUM drain; alternate engines
                dst = y[:, (bb - bb0) * PB:(bb - bb0 + 1) * PB]
                if bb % 2 == 0:
                    nc.scalar.activation(
                        out=dst, in_=ps[:],
                        func=mybir.ActivationFunctionType.Relu)
                else:
                    nc.vector.tensor_scalar_max(dst, ps[:], 0.0)
            nc.sync.dma_start(
                out=out_d[:, bb0 * PB:(bb0 + nb) * PB], in_=y[:])
            xc = xsched.get(gi)
            if xc is not None:
                nc.sync.dma_start(
                    out=xts[xc][:],
                    in_=x_d[:, 2 * bounds[xc]:2 * bounds[xc + 1]])
            bb0 += nb

    nc.compile()
    return nc


def _ensure_ntff_hook():
    """This image's antenv lacks axon_hooks; synthesize it so trace=True can
    drive NTFF profiling via the injected libaxon_pjrt.so."""
    import sys
    import types
    try:
        import antenv.axon_hooks  # noqa: F401
        return True
    except ImportError:
        pass
    try:
        import antenv
        from trn_agent_boot.trn_boot import _ntff_profile_via_ctypes
    except ImportError:
        return False
    mod = types.ModuleType("antenv.axon_hooks")
    holder = {}
    mod.set_axon_ntff_profile_hook = lambda h: holder.__setitem__("h", h)
    mod.get_axon_ntff_profile_hook = lambda: holder.get("h")
    sys.modules["antenv.axon_hooks"] = mod
    antenv.axon_hooks = mod
    try:
        h = _ntff_profile_via_ctypes("/opt/axon/libaxon_pjrt.so")
    except OSError:
        h = None
    if h is not None:
        mod.set_axon_ntff_profile_hook(h)
    return True


def kernel(**inputs):
    global LAST_RESULTS
    import ml_dtypes
    from concourse.bass_utils import run_bass_kernel_spmd

    bf16 = ml_dtypes.bfloat16
    feats = np.asarray(inputs["feats"], dtype=np.float32)
    weight = np.asarray(inputs["weight"], dtype=np.float32)
    gather_idx = np.asarray(inputs["gather_idx"], dtype=np.int64)
    scatter_idx = np.asarray(inputs["scatter_idx"], dtype=np.int64)
    n_out = int(inputs["n_out"])
    assert feats.shape == (N_IN, C_IN) and weight.shape == (K, C_IN, C_OUT)
    assert n_out == N_OUT

    # Per output row j: its unique (parent, koff) match from the match lists.
    par_j = np.zeros(N_OUT, dtype=np.int64)
    koff_j = np.zeros(N_OUT, dtype=np.int64)
    covered = np.zeros(N_OUT, dtype=bool)
    for k in range(K):
        s = scatter_idx[k]
        g = gather_idx[k]
        valid = (s < N_OUT) & (g < N_IN)
        par_j[s[valid]] = g[valid]
        koff_j[s[valid]] = k
        covered[s[valid]] = True

    # Class of each parent = bitmask of its matched offsets (exactly 4 set).
    cls = np.zeros(N_IN, dtype=np.int64)
    np.bitwise_or.at(cls, par_j[covered], np.int64(1) << koff_j[covered])
    popc = np.zeros(N_IN, dtype=np.int64)
    for k in range(K):
        popc += (cls >> k) & 1
    assert (popc == CHILDREN).all(), "every parent must match exactly 4 offsets"
    lut = np.full(256, -1, dtype=np.int64)
    for i, m in enumerate(_CLASS_MASKS):
        lut[m] = i
    crank = lut[cls]
    assert (crank >= 0).all()

    # Shard parents per-class round-robin across cores: member m of class g
    # goes to core m%8 at padded slot off[g] + m//8, so per-core class
    # counts differ by at most 1 and the shared padded layout wastes ~0.3%
    # instead of ~10% (core-range sharding). The host-side selection below
    # may read any core's slab, so sharding is free to permute parents.
    order_g = np.argsort(crank, kind="stable")
    sorted_ranks = crank[order_g]
    n_g = np.bincount(crank, minlength=NCLS)
    grp_start = np.zeros(NCLS, dtype=np.int64)
    grp_start[1:] = np.cumsum(n_g)[:-1]
    m_idx = np.arange(N_IN) - grp_start[sorted_ranks]
    core_of = np.empty(N_IN, dtype=np.int64)
    core_of[order_g] = m_idx % NCORES
    cnt_max = -(-n_g // NCORES)
    NP, off, bounds, pieces, T = _layout(cnt_max)
    T512 = -(-T // PB) * PB
    pp_all = np.empty(N_IN, dtype=np.int64)
    pp_all[order_g] = off[sorted_ranks] + m_idx // NCORES

    # Token index of every padded x slot, per offset (device piece order);
    # identical for all cores.
    tokmap = np.full((K, NP), -1, dtype=np.int64)
    for k, ch, loff, tok, n in pieces:
        xoff = bounds[ch] + loff
        tokmap[k, xoff:xoff + n] = np.arange(tok, tok + n)

    # Per-core bf16 operand layout.
    w2 = np.ascontiguousarray(
        weight.reshape(2, 4, 2, 128, C_OUT).transpose(3, 0, 2, 1, 4)
    ).reshape(128, 2, K, C_OUT).astype(bf16)
    in_maps = []
    for c in range(NCORES):
        mine = core_of == c
        f = np.zeros((NP, C_IN), dtype=np.float32)
        f[pp_all[mine]] = feats[mine]
        fh = f.reshape(NP, 2, 128).transpose(2, 1, 0)   # [p, h, i]
        x = np.empty((128, 2 * NP), dtype=np.float32)
        for a, b in zip(bounds[:-1], bounds[1:]):
            x[:, 2 * a:a + b] = fh[:, 0, a:b]
            x[:, a + b:2 * b] = fh[:, 1, a:b]
        in_maps.append({"x": x.astype(bf16), "w": w2})

    nc = _build_program(tuple(int(v) for v in cnt_max))
    trace = bool(int(os.environ.get("KERNEL_TRACE", "0")))
    if trace:
        trace = _ensure_ntff_hook()
    res = run_bass_kernel_spmd(nc, in_maps, list(range(NCORES)), trace=trace)
    LAST_RESULTS = res

    # Unshard: token -> output row inverse permutation (pure numpy).
    a_all = np.stack([np.asarray(res.results[c]["out"])
                      for c in range(NCORES)])          # [8, 128, T512] bf16
    out = np.zeros((N_OUT, C_OUT), dtype=np.float32)
    pj = par_j[covered]
    tok = tokmap[koff_j[covered], pp_all[pj]]
    assert (tok >= 0).all()
    out[covered] = a_all[core_of[pj], :, tok].astype(np.float32)
    return out
